# revision 10
# baseline (speedup 1.0000x reference)
"""GNN message passing (gather + weighted scatter-add) on 8 Trainium2 cores, v2.

out[n, f] = sum over edges e with dst[e]==n of edge_weight[e] * x[src[e], f]

Architecture (driven by measured per-instruction dispatch costs of ~30-120us
on this runtime — total instruction count is everything):
  - Destination-shard: core c owns output nodes [c*12500, (c+1)*12500).
  - x lives fully SBUF-resident, fp16, transposed + quarter-partitioned:
    xT4[p, n, l] = x[(p//32)*25000 + n, 2*(p%32) + l]  -> [128, 25000, 2],
    100KB/partition. int16 ap_gather indices stay < 25000.
  - Edges packed into 8-slot rows per dst node (ceil(deg/8) rows/node).
    Slot j: ap_gather pulls x columns for src_j into every partition
    (each 16-partition group uses its own index: the src quarter that
    group holds, else a dummy); host-baked weights wt4[p, j] =
    w_j * (quarter(p) == quarter(src_j)) kill the wrong-quarter copies.
  - Per chunk (~6-7k slots): 1 ap_gather (gpsimd) + 1 broadcast multiply
    (DVE) + 1 strided row-reduce (DVE) + wt/rowsum DMAs. ~30 chunks/core,
    ~130 instructions total.
  - Row sums [128=(pair,quarter), rows, 2] stream to DRAM; the host sums
    the <=4 quarter partials and ceil(deg/8) row partials per node (O(N)
    work) and re-interleaves features.
"""

import math
import numpy as np

N = 100000
E = 1000000
F = 64
NCORES = 8
NPC = N // NCORES            # 12500 dst nodes per core
Q = 4                        # x quarters (int16 index limit)
NQ = N // Q                  # 25000
NQ1 = NQ + 1                 # +1 zero-sentinel column (gathered by dummies)
L = 4                        # slots per row
CHUNK = 8704                 # slots per chunk (multiple of 128)

MULT_MODE = "dve"            # "dve" | "gpsimd" | "split" (alternate)
REPEAT = 1                   # device-body repetitions (timing amplification)


def pack_host(x, edge_weight, edge_index):
    src = np.asarray(edge_index[0], dtype=np.int64)
    dst = np.asarray(edge_index[1], dtype=np.int64)
    w = np.asarray(edge_weight, dtype=np.float32)

    xpair = np.ascontiguousarray(
        x.astype(np.float16).reshape(N, 32, 2).transpose(1, 0, 2)
    )  # [32, N, 2]
    xt4 = np.zeros((128, NQ1, 2), dtype=np.float16)
    xt4[:, 1:, :] = (
        xpair.reshape(32, Q, NQ, 2).transpose(1, 0, 2, 3).reshape(128, NQ, 2)
    )  # partition p = (q = p//32, pair = p%32); column 0 stays zero (sentinel)

    core = dst // NPC
    cores = []
    for c in range(NCORES):
        sel = core == c
        es = src[sel]
        ed = dst[sel] - c * NPC
        ew = w[sel]
        order = np.argsort(ed, kind="stable")
        es, ed, ew = es[order], ed[order], ew[order]

        deg = np.bincount(ed, minlength=NPC)
        nrows_per_node = np.maximum((deg + L - 1) // L, 1)
        nrows = int(nrows_per_node.sum())
        # chunk layout in rows (CHUNK/L rows per chunk), pad rows to fill
        rows_per_chunk = CHUNK // L
        nchunks = math.ceil(nrows / rows_per_chunk)
        nrows_pad = nchunks * rows_per_chunk
        nslots = nrows_pad * L

        row_node = np.zeros(nrows_pad, dtype=np.int64)  # node of each row
        # rows in node order
        row_node[:nrows] = np.repeat(np.arange(NPC), nrows_per_node)
        row_node[nrows:] = 0  # pad rows -> node 0 with zero weight

        slot_src = np.zeros(nslots, dtype=np.int64)
        slot_w = np.zeros(nslots, dtype=np.float32)
        # edge positions: node n's edges go into its rows' slots in order
        row_start = np.zeros(NPC + 1, dtype=np.int64)
        row_start[1:] = np.cumsum(nrows_per_node)
        node_edge_start = np.zeros(NPC + 1, dtype=np.int64)
        node_edge_start[1:] = np.cumsum(deg)
        # position of edge within its node  (edges are dst-sorted)
        epos = np.arange(len(ed)) - node_edge_start[ed]
        slot_idx = row_start[ed] * L + epos
        slot_src[slot_idx] = es
        slot_w[slot_idx] = ew

        sq = slot_src // NQ          # quarter of each slot's src
        sl = (slot_src % NQ + 1).astype(np.int16)  # 1-based; 0 = zero sentinel

        # ap_gather index table [128, nslots/16] int16, per-16-partition group:
        # the group holding quarter q gathers its slots' rows; others gather 0s
        idx16 = np.zeros((128, nslots // 16), dtype=np.int16)
        slocal = sl.reshape(nslots // 16, 16).T  # [16, s]
        squar = sq.reshape(nslots // 16, 16).T
        for g in range(8):
            gq = g // 2
            idx16[g * 16:(g + 1) * 16, :] = np.where(squar == gq, slocal, 0)

        # per-slot weights [1, nslots] fp16 (quarter masking is redundant:
        # wrong-quarter partitions gather the zero sentinel column)
        w1 = slot_w.astype(np.float16)[None, :]

        cores.append(dict(
            idx16=idx16, w1=w1, nrows=nrows, nrows_pad=nrows_pad,
            nslots=nslots, nchunks=nchunks, row_node=row_node,
        ))
    maxchunks = max(c["nchunks"] for c in cores)
    # pad all cores to identical chunk count (single SPMD program)
    for c in cores:
        if c["nchunks"] < maxchunks:
            extra = (maxchunks - c["nchunks"]) * CHUNK
            c["idx16"] = np.concatenate(
                [c["idx16"], np.zeros((128, extra // 16), np.int16)], axis=1)
            c["w1"] = np.concatenate(
                [c["w1"], np.zeros((1, extra), np.float16)], axis=1)
            pad_rows = extra // L
            c["row_node"] = np.concatenate(
                [c["row_node"], np.zeros(pad_rows, np.int64)])
            c["nchunks"] = maxchunks
            c["nrows_pad"] += pad_rows
            c["nslots"] += extra
    return xt4, cores, maxchunks


def emulate_core(xt4, core):
    """Numpy emulation of the device program for one core."""
    nch = core["nchunks"]
    idx16, w1 = core["idx16"], core["w1"]
    rows_out = np.zeros((128, core["nrows_pad"], 2), dtype=np.float16)
    for ch in range(nch):
        s0 = ch * CHUNK
        g = np.zeros((128, CHUNK, 2), dtype=np.float16)
        for grp in range(8):
            idxs = idx16[grp * 16:(grp + 1) * 16, s0 // 16:(s0 + CHUNK) // 16]
            flat = idxs.T.reshape(-1).astype(np.int64)  # slot order
            g[grp * 16:(grp + 1) * 16] = xt4[grp * 16:(grp + 1) * 16, flat, :]
        wc = w1[0, s0:s0 + CHUNK, None].astype(np.float32)
        gw = (g.astype(np.float32) * wc[None]).astype(np.float16)
        r = gw.astype(np.float32).reshape(128, CHUNK // L, L, 2).sum(axis=2)
        rows_out[:, s0 // L:(s0 + CHUNK) // L, :] = r.astype(np.float16)
    return rows_out


def combine_host(rows_out, core):
    """rows [128=(q,pair), rows, 2] fp16 -> [12500, 64] f32 for one core."""
    r = rows_out.astype(np.float32)  # [128, R, 2]
    rq = r.reshape(Q, 32, -1, 2).sum(axis=0)  # [32, R, 2]
    out = np.zeros((NPC, 32, 2), dtype=np.float32)
    np.add.at(out, core["row_node"], rq.transpose(1, 0, 2))
    return out.reshape(NPC, F)


WAIT_CAPS = {"InstEventSemaphore": 8}


def split_excess_waits(nc):
    """Walrus only encodes one sync wait per instruction; move the excess
    onto standalone InstEventSemaphore instructions placed just before."""
    import concourse.mybir as mybir
    n = 0
    for f in nc.m.functions:
        for bb in f.blocks:
            eng_ids = {}
            new = []
            for ins in bb.instructions:
                si = ins.sync_info
                waits = list(si.on_wait) if (si is not None and si.on_wait) else []
                cap = WAIT_CAPS.get(type(ins).__name__, 1)
                if len(waits) > cap:
                    excess, keep = waits[:-cap], waits[-cap:]
                    if ins.engine not in eng_ids:
                        eng_ids[ins.engine] = 245 + len(eng_ids)
                    sem_id = eng_ids[ins.engine]
                    sem_name = f"esw_scratch_{sem_id}"
                    for wchunk in [excess[i:i + 1] for i in range(len(excess))]:
                        n += 1
                        upd = mybir.SyncUpdate(
                            sync_type="semaphore", id=sem_id, ant_name=sem_name,
                            update_mode="sem-add-imm", update_value=0,
                        )
                        es = mybir.InstEventSemaphore(
                            name=f"ESW-{n}-{ins.name}",
                            engine=ins.engine,
                            ins=[], outs=[],
                            sync_info=mybir.SyncInfo(on_wait=wchunk, on_update=[upd]),
                        )
                        new.append(es)
                    si.on_wait = keep
                new.append(ins)
            bb.instructions = new
    return n


_walrus_patched = False


def patch_walrus_dge():
    global _walrus_patched
    if _walrus_patched:
        return
    import concourse.bass_utils as bu
    orig = bu.run_command

    def run_command_dge(argv, **kw):
        argv = list(argv)
        if argv and "walrus_driver" in str(argv[0]) and not any(
                str(a).startswith("--dge-levels") for a in argv):
            argv.append("--dge-levels=vector_dynamic_offsets")
        return orig(argv, **kw)

    bu.run_command = run_command_dge
    _walrus_patched = True


def build_bass(nchunks, nslots):
    import concourse.bass as bass
    import concourse.mybir as mybir
    import concourse.tile as tile
    from concourse import library_config
    from concourse.library_overlay import lower_extended_insts

    patch_walrus_dge()
    f16, i16 = mybir.dt.float16, mybir.dt.int16
    nrows_pad = nslots // L

    nc = bass.Bass("TRN2")
    xt4_d = nc.dram_tensor("xt4", [128, NQ1, 2], f16, kind="ExternalInput")
    idx_d = nc.dram_tensor("idx16", [128, nslots // 16], i16, kind="ExternalInput")
    w1_d = nc.dram_tensor("w1", [1, nslots], f16, kind="ExternalInput")
    rows_d = nc.dram_tensor("rows", [128, nrows_pad, 2], f16, kind="ExternalOutput")

    with nc.allow_low_precision("fp16 8-term row sums; host combines in f32"):
      with tile.TileContext(nc, pool_alloc_mode="queue") as tc:
        with (
            tc.tile_pool(name="const", bufs=1) as constp,
            tc.tile_pool(name="g", bufs=2) as gp,
            tc.tile_pool(name="wt", bufs=1) as wtp,
            tc.tile_pool(name="ix", bufs=2) as ixp,
            tc.tile_pool(name="rs", bufs=2) as rsp,
        ):
            xt4_sb = constp.tile([128, NQ1, 2], f16, tag="xt4")
            nc.sync.dma_start(xt4_sb[:], xt4_d[:])
            nc.gpsimd.load_library(library_config.ap_gather)

            for _rep in range(REPEAT):
                for ch in range(nchunks):
                    s0 = ch * CHUNK
                    idxc = ixp.tile([128, CHUNK // 16], i16, tag="ix")
                    nc.scalar.dma_start(
                        idxc[:], idx_d[:, s0 // 16:(s0 + CHUNK) // 16])
                    g = gp.tile([128, CHUNK, 2], f16, tag="g")
                    nc.gpsimd.ap_gather(
                        g[:], xt4_sb[:], idxc[:],
                        channels=128, num_elems=NQ1, d=2, num_idxs=CHUNK)
                    wtc = wtp.tile([128, CHUNK], f16, tag="wt")
                    nc.scalar.dma_start(
                        wtc[:], w1_d[0:1, s0:s0 + CHUNK].broadcast_to([128, CHUNK]))
                    if MULT_MODE == "dve":
                        mult_eng = nc.vector
                    elif MULT_MODE == "gpsimd":
                        mult_eng = nc.gpsimd
                    else:
                        mult_eng = nc.vector if ch % 2 == 0 else nc.gpsimd
                    mult_eng.tensor_tensor(
                        out=g[:], in0=g[:],
                        in1=wtc[:].unsqueeze(2).broadcast_to([128, CHUNK, 2]),
                        op=mybir.AluOpType.mult)
                    rs = rsp.tile([128, CHUNK // L, 2], f16, tag="rs")
                    nc.vector.tensor_reduce(
                        out=rs[:],
                        in_=g[:].rearrange("p (r k) two -> p r two k", k=L),
                        axis=mybir.AxisListType.X, op=mybir.AluOpType.add)
                    nc.sync.dma_start(
                        rows_d[:, s0 // L:(s0 + CHUNK) // L, :], rs[:])

    lower_extended_insts(nc)
    split_excess_waits(nc)
    return nc


def kernel(x, edge_weight, edge_index, num_nodes):
    x = np.ascontiguousarray(np.asarray(x, dtype=np.float32))
    xt4, cores, nchunks = pack_host(x, edge_weight, edge_index)
    nslots = cores[0]["nslots"]
    nc = build_bass(nchunks, nslots)
    in_maps = [
        {"xt4": xt4, "idx16": c["idx16"], "w1": c["w1"]} for c in cores
    ]
    from concourse.bass_utils import run_bass_kernel_spmd
    res = run_bass_kernel_spmd(nc, in_maps, core_ids=list(range(NCORES)))
    outs = [combine_host(res.results[c]["rows"], cores[c])
            for c in range(NCORES)]
    return np.concatenate(outs, axis=0).astype(np.float32)


# revision 11
# speedup vs baseline: 5.2494x; 5.2494x over previous
"""GNN message passing (gather + weighted scatter-add) on 8 Trainium2 cores, v2.

out[n, f] = sum over edges e with dst[e]==n of edge_weight[e] * x[src[e], f]

Architecture (driven by measured per-instruction dispatch costs of ~30-120us
on this runtime — total instruction count is everything):
  - Destination-shard: core c owns output nodes [c*12500, (c+1)*12500).
  - x lives fully SBUF-resident, fp16, transposed + quarter-partitioned:
    xT4[p, n, l] = x[(p//32)*25000 + n, 2*(p%32) + l]  -> [128, 25000, 2],
    100KB/partition. int16 ap_gather indices stay < 25000.
  - Edges packed into 8-slot rows per dst node (ceil(deg/8) rows/node).
    Slot j: ap_gather pulls x columns for src_j into every partition
    (each 16-partition group uses its own index: the src quarter that
    group holds, else a dummy); host-baked weights wt4[p, j] =
    w_j * (quarter(p) == quarter(src_j)) kill the wrong-quarter copies.
  - Per chunk (~6-7k slots): 1 ap_gather (gpsimd) + 1 broadcast multiply
    (DVE) + 1 strided row-reduce (DVE) + wt/rowsum DMAs. ~30 chunks/core,
    ~130 instructions total.
  - Row sums [128=(pair,quarter), rows, 2] stream to DRAM; the host sums
    the <=4 quarter partials and ceil(deg/8) row partials per node (O(N)
    work) and re-interleaves features.
"""

import math
import numpy as np

N = 100000
E = 1000000
F = 64
NCORES = 8
NPC = N // NCORES            # 12500 dst nodes per core
Q = 4                        # x quarters (int16 index limit)
NQ = N // Q                  # 25000
NQ1 = NQ + 1                 # +1 zero-sentinel column (gathered by dummies)
L = 4                        # slots per row
CHUNK = 7680                 # slots per chunk (multiple of 128)

MULT_MODE = "dve"            # "dve" | "gpsimd" | "split" (alternate)
REPEAT = 1                   # device-body repetitions (timing amplification)


def pack_host(x, edge_weight, edge_index):
    src = np.asarray(edge_index[0], dtype=np.int64)
    dst = np.asarray(edge_index[1], dtype=np.int64)
    w = np.asarray(edge_weight, dtype=np.float32)

    xpair = np.ascontiguousarray(
        x.astype(np.float16).reshape(N, 32, 2).transpose(1, 0, 2)
    )  # [32, N, 2]
    xt4 = np.zeros((128, NQ1, 2), dtype=np.float16)
    xt4[:, 1:, :] = (
        xpair.reshape(32, Q, NQ, 2).transpose(1, 0, 2, 3).reshape(128, NQ, 2)
    )  # partition p = (q = p//32, pair = p%32); column 0 stays zero (sentinel)

    core = dst // NPC
    cores = []
    for c in range(NCORES):
        sel = core == c
        es = src[sel]
        ed = dst[sel] - c * NPC
        ew = w[sel]
        order = np.argsort(ed, kind="stable")
        es, ed, ew = es[order], ed[order], ew[order]

        deg = np.bincount(ed, minlength=NPC)
        nrows_per_node = np.maximum((deg + L - 1) // L, 1)
        nrows = int(nrows_per_node.sum())
        # chunk layout in rows (CHUNK/L rows per chunk), pad rows to fill
        rows_per_chunk = CHUNK // L
        nchunks = math.ceil(nrows / rows_per_chunk)
        nrows_pad = nchunks * rows_per_chunk
        nslots = nrows_pad * L

        row_node = np.zeros(nrows_pad, dtype=np.int64)  # node of each row
        # rows in node order
        row_node[:nrows] = np.repeat(np.arange(NPC), nrows_per_node)
        row_node[nrows:] = 0  # pad rows -> node 0 with zero weight

        slot_src = np.zeros(nslots, dtype=np.int64)
        slot_w = np.zeros(nslots, dtype=np.float32)
        # edge positions: node n's edges go into its rows' slots in order
        row_start = np.zeros(NPC + 1, dtype=np.int64)
        row_start[1:] = np.cumsum(nrows_per_node)
        node_edge_start = np.zeros(NPC + 1, dtype=np.int64)
        node_edge_start[1:] = np.cumsum(deg)
        # position of edge within its node  (edges are dst-sorted)
        epos = np.arange(len(ed)) - node_edge_start[ed]
        slot_idx = row_start[ed] * L + epos
        slot_src[slot_idx] = es
        slot_w[slot_idx] = ew

        sq = slot_src // NQ          # quarter of each slot's src
        sl = (slot_src % NQ + 1).astype(np.int16)  # 1-based; 0 = zero sentinel

        # ap_gather index table [128, nslots/16] int16, per-16-partition group:
        # the group holding quarter q gathers its slots' rows; others gather 0s
        idx16 = np.zeros((128, nslots // 16), dtype=np.int16)
        slocal = sl.reshape(nslots // 16, 16).T  # [16, s]
        squar = sq.reshape(nslots // 16, 16).T
        for g in range(8):
            gq = g // 2
            idx16[g * 16:(g + 1) * 16, :] = np.where(squar == gq, slocal, 0)

        # per-slot weights [1, nslots] fp16 (quarter masking is redundant:
        # wrong-quarter partitions gather the zero sentinel column)
        w1 = slot_w.astype(np.float16)[None, :]

        cores.append(dict(
            idx16=idx16, w1=w1, nrows=nrows, nrows_pad=nrows_pad,
            nslots=nslots, nchunks=nchunks, row_node=row_node,
        ))
    maxchunks = max(c["nchunks"] for c in cores)
    # pad all cores to identical chunk count (single SPMD program)
    for c in cores:
        if c["nchunks"] < maxchunks:
            extra = (maxchunks - c["nchunks"]) * CHUNK
            c["idx16"] = np.concatenate(
                [c["idx16"], np.zeros((128, extra // 16), np.int16)], axis=1)
            c["w1"] = np.concatenate(
                [c["w1"], np.zeros((1, extra), np.float16)], axis=1)
            pad_rows = extra // L
            c["row_node"] = np.concatenate(
                [c["row_node"], np.zeros(pad_rows, np.int64)])
            c["nchunks"] = maxchunks
            c["nrows_pad"] += pad_rows
            c["nslots"] += extra
    return xt4, cores, maxchunks


def emulate_core(xt4, core):
    """Numpy emulation of the device program for one core."""
    nch = core["nchunks"]
    idx16, w1 = core["idx16"], core["w1"]
    rows_out = np.zeros((128, core["nrows_pad"], 2), dtype=np.float16)
    for ch in range(nch):
        s0 = ch * CHUNK
        g = np.zeros((128, CHUNK, 2), dtype=np.float16)
        for grp in range(8):
            idxs = idx16[grp * 16:(grp + 1) * 16, s0 // 16:(s0 + CHUNK) // 16]
            flat = idxs.T.reshape(-1).astype(np.int64)  # slot order
            g[grp * 16:(grp + 1) * 16] = xt4[grp * 16:(grp + 1) * 16, flat, :]
        wc = w1[0, s0:s0 + CHUNK, None].astype(np.float32)
        gw = (g.astype(np.float32) * wc[None]).astype(np.float16)
        r = gw.astype(np.float32).reshape(128, CHUNK // L, L, 2).sum(axis=2)
        rows_out[:, s0 // L:(s0 + CHUNK) // L, :] = r.astype(np.float16)
    return rows_out


def combine_host(rows_out, core):
    """rows [128=(q,pair), rows, 2] fp16 -> [12500, 64] f32 for one core."""
    r = rows_out.astype(np.float32)  # [128, R, 2]
    rq = r.reshape(Q, 32, -1, 2).sum(axis=0)  # [32, R, 2]
    out = np.zeros((NPC, 32, 2), dtype=np.float32)
    np.add.at(out, core["row_node"], rq.transpose(1, 0, 2))
    return out.reshape(NPC, F)


WAIT_CAPS = {"InstEventSemaphore": 8}


def split_excess_waits(nc):
    """Walrus only encodes one sync wait per instruction; move the excess
    onto standalone InstEventSemaphore instructions placed just before."""
    import concourse.mybir as mybir
    n = 0
    for f in nc.m.functions:
        for bb in f.blocks:
            eng_ids = {}
            new = []
            for ins in bb.instructions:
                si = ins.sync_info
                waits = list(si.on_wait) if (si is not None and si.on_wait) else []
                cap = WAIT_CAPS.get(type(ins).__name__, 1)
                if len(waits) > cap:
                    excess, keep = waits[:-cap], waits[-cap:]
                    if ins.engine not in eng_ids:
                        eng_ids[ins.engine] = 245 + len(eng_ids)
                    sem_id = eng_ids[ins.engine]
                    sem_name = f"esw_scratch_{sem_id}"
                    for wchunk in [excess[i:i + 1] for i in range(len(excess))]:
                        n += 1
                        upd = mybir.SyncUpdate(
                            sync_type="semaphore", id=sem_id, ant_name=sem_name,
                            update_mode="sem-add-imm", update_value=0,
                        )
                        es = mybir.InstEventSemaphore(
                            name=f"ESW-{n}-{ins.name}",
                            engine=ins.engine,
                            ins=[], outs=[],
                            sync_info=mybir.SyncInfo(on_wait=wchunk, on_update=[upd]),
                        )
                        new.append(es)
                    si.on_wait = keep
                new.append(ins)
            bb.instructions = new
    return n


_walrus_patched = False


def patch_walrus_dge():
    global _walrus_patched
    if _walrus_patched:
        return
    import concourse.bass_utils as bu
    orig = bu.run_command

    def run_command_dge(argv, **kw):
        argv = list(argv)
        if argv and "walrus_driver" in str(argv[0]) and not any(
                str(a).startswith("--dge-levels") for a in argv):
            argv.append("--dge-levels=vector_dynamic_offsets")
        return orig(argv, **kw)

    bu.run_command = run_command_dge
    _walrus_patched = True


def build_bass(nchunks, nslots):
    import concourse.bass as bass
    import concourse.mybir as mybir
    import concourse.tile as tile
    from concourse import library_config
    from concourse.library_overlay import lower_extended_insts

    patch_walrus_dge()
    f16, i16 = mybir.dt.float16, mybir.dt.int16
    nrows_pad = nslots // L

    nc = bass.Bass("TRN2")
    xt4_d = nc.dram_tensor("xt4", [128, NQ1, 2], f16, kind="ExternalInput")
    idx_d = nc.dram_tensor("idx16", [128, nslots // 16], i16, kind="ExternalInput")
    w1_d = nc.dram_tensor("w1", [1, nslots], f16, kind="ExternalInput")
    rows_d = nc.dram_tensor("rows", [128, nrows_pad, 2], f16, kind="ExternalOutput")

    with nc.allow_low_precision("fp16 8-term row sums; host combines in f32"):
      with tile.TileContext(nc, pool_alloc_mode="queue") as tc:
        with (
            tc.tile_pool(name="const", bufs=1) as constp,
            tc.tile_pool(name="g", bufs=2) as gp,
            tc.tile_pool(name="wt", bufs=1) as wtp,
            tc.tile_pool(name="rs", bufs=2) as rsp,
        ):
            xt4_sb = constp.tile([128, NQ1, 2], f16, tag="xt4")
            nc.sync.dma_start(xt4_sb[:], xt4_d[:])
            idx_sb = constp.tile([128, nslots // 16], i16, tag="idx")
            nc.sync.dma_start(idx_sb[:], idx_d[:])
            nc.gpsimd.load_library(library_config.ap_gather)

            for _rep in range(REPEAT):
                for ch in range(nchunks):
                    s0 = ch * CHUNK
                    g = gp.tile([128, CHUNK, 2], f16, tag="g")
                    nc.gpsimd.ap_gather(
                        g[:], xt4_sb[:], idx_sb[:, s0 // 16:(s0 + CHUNK) // 16],
                        channels=128, num_elems=NQ1, d=2, num_idxs=CHUNK)
                    wtc = wtp.tile([128, CHUNK], f16, tag="wt")
                    nc.scalar.dma_start(
                        wtc[:], w1_d[0:1, s0:s0 + CHUNK].broadcast_to([128, CHUNK]))
                    if MULT_MODE == "dve":
                        mult_eng = nc.vector
                    elif MULT_MODE == "gpsimd":
                        mult_eng = nc.gpsimd
                    else:
                        mult_eng = nc.vector if ch % 2 == 0 else nc.gpsimd
                    mult_eng.tensor_tensor(
                        out=g[:], in0=g[:],
                        in1=wtc[:].unsqueeze(2).broadcast_to([128, CHUNK, 2]),
                        op=mybir.AluOpType.mult)
                    rs = rsp.tile([128, CHUNK // L, 2], f16, tag="rs")
                    nc.vector.tensor_reduce(
                        out=rs[:],
                        in_=g[:].rearrange("p (r k) two -> p r two k", k=L),
                        axis=mybir.AxisListType.X, op=mybir.AluOpType.add)
                    nc.sync.dma_start(
                        rows_d[:, s0 // L:(s0 + CHUNK) // L, :], rs[:])

    lower_extended_insts(nc)
    split_excess_waits(nc)
    return nc


def kernel(x, edge_weight, edge_index, num_nodes):
    x = np.ascontiguousarray(np.asarray(x, dtype=np.float32))
    xt4, cores, nchunks = pack_host(x, edge_weight, edge_index)
    nslots = cores[0]["nslots"]
    nc = build_bass(nchunks, nslots)
    in_maps = [
        {"xt4": xt4, "idx16": c["idx16"], "w1": c["w1"]} for c in cores
    ]
    from concourse.bass_utils import run_bass_kernel_spmd
    res = run_bass_kernel_spmd(nc, in_maps, core_ids=list(range(NCORES)))
    outs = [combine_host(res.results[c]["rows"], cores[c])
            for c in range(NCORES)]
    return np.concatenate(outs, axis=0).astype(np.float32)
